# revision 11
# baseline (speedup 1.0000x reference)
"""CoAttention kernel for Trainium2 (8 NeuronCores, data-parallel over batch).

Math (per sample): ta = relu(seq_a @ W + b), tb likewise.  The reference
mean-pools the [N, rv_len, M] affinity before softmax, and mean-pooling
commutes with the dot product:

    atob_scores[n, l] = mean_m( ta[n,l,:] . tb_all_tokens[m,:] )
                      = ta[n,l,:] . mean_m( tb_all_tokens[m,:] )

so each side only needs a dot with the *other side's per-sample mean
feature vector* — the 52M-element affinity tensor is never materialized.

Memory regime: HBM-bound on the seq reads, so the host pre-converts seq
(and W) to fp16, halving DMA traffic; end-to-end error stays ~4e-3 vs
the fp32 reference.  Seq streams in as 6 full-span DMAs per iteration
(one per side x contraction-chunk) split across the two HWDGE queues
(SP + Act); gpsimd's software DGE issues nothing.

The FC runs as fp16 PE matmuls with W stationary; taT [hdim, tokens]
stays resident in SBUF as fp16 (enables the DVE 2x perf mode on the
weighted-sum multiply).  Scores are per-review matvecs with the taT
block stationary and the other side's mean as a 1-col moving operand,
landing transposed in a [128, 40] PSUM tile -> free-size-40 act-engine
evacuation + one PE transpose restores the softmax layout.

The tail (scores/softmax/weighted-sum) processes SAMPLE PAIRS: all
softmax ops run on [40,128] tiles, masking is a single tensor_tensor
add of a host-prebuilt additive bias (0 / -1e9), and the weighted sum
does one [128,2560] broadcast+in-place-multiply+segmented-reduce per
side per pair — halving tail instruction count vs per-sample tiles.
"""
import sys

sys.path.insert(0, "/opt/trn_rl_repo")

import numpy as np

import concourse.bacc as bacc
import concourse.tile as tile
from concourse import mybir

# Problem shape (hardcoded per contest contract)
BZ, RV, RL, DIN, DH = 32, 10, 128, 300, 128
NCORES = 8
BPC = BZ // NCORES            # samples per core: 4
PAIRS = BPC // 2              # tail processes samples in pairs: 2
TPC = BPC * RV * RL           # tokens per core per side: 5120
TPS = RV * RL                 # tokens per sample: 1280
TPP = 2 * TPS                 # tokens per pair: 2560
RPC = BPC * RV                # reviews per core: 40
NEG_INF = -1e9

f32 = mybir.dt.float32
f16 = mybir.dt.float16
i32 = mybir.dt.int32
AF = mybir.ActivationFunctionType
AX = mybir.AxisListType

# d-chunks of the contraction dim (K <= 128)
DCH = [(0, 128), (128, 128), (256, 44)]
# free-dim chunks of one sample's tokens (N <= 512 per PSUM bank)
NCH = [(0, 512), (512, 512), (1024, 256)]

_CACHE = {}


def _build(iters=1, serial=False, loop_n=0, stage=3):
    nc = bacc.Bacc("TRN2", target_bir_lowering=False, debug=False)

    sqt = {s: nc.dram_tensor(f"sqt_{s}", [DIN, TPC], f16, kind="ExternalInput")
           for s in "ab"}
    mbias_d = nc.dram_tensor("mbias", [4 * RV, PAIRS * RL], f32,
                             kind="ExternalInput")
    w_d = nc.dram_tensor("w", [DIN, DH], f16, kind="ExternalInput")
    bias_d = nc.dram_tensor("bias", [DH, 1], f32, kind="ExternalInput")
    ident_d = nc.dram_tensor("ident", [DH, DH], f32, kind="ExternalInput")

    out_v = {s: nc.dram_tensor(f"out_{s}", [RPC, DH], f32, kind="ExternalOutput")
             for s in "ab"}
    out_w = {s: nc.dram_tensor(f"outw_{s}", [RPC, RL], f32, kind="ExternalOutput")
             for s in "ab"}

    import contextlib
    outer_tc = tile.TileContext(nc) if not serial else None
    with (outer_tc if outer_tc is not None else contextlib.nullcontext()):
      for it_ in range(iters):
        pfx = f"i{it_}_" if iters > 1 else ""
        with (
            tile.TileContext(nc) if serial else contextlib.nullcontext()
        ) as maybe_tc:
          tc = maybe_tc if serial else outer_tc
          with (
            tc.For_i(0, loop_n, 1) if loop_n else contextlib.nullcontext()
          ):
           with (
            tc.tile_pool(name=pfx + "cst", bufs=1) as cst,
            tc.tile_pool(name=pfx + "seq", bufs=2) as seqp,
            tc.tile_pool(name=pfx + "big", bufs=1) as bigp,
            tc.tile_pool(name=pfx + "sm", bufs=2) as smp_pool,
            tc.tile_pool(name=pfx + "ps", bufs=2, space="PSUM") as ps,
        ):
            # full-span seq tiles: one DMA per (side, chunk) covering all 4
            # samples — 6 big DMAs/iter, split across the two HWDGE queues.
            sq = {}
            seq_eng = {("b", 0): nc.sync, ("a", 0): nc.scalar,
                       ("b", 1): nc.sync, ("a", 1): nc.scalar,
                       ("b", 2): nc.sync, ("a", 2): nc.scalar}
            for c, (d0, dw) in enumerate(DCH):
                for s in ("b", "a"):
                    sq[(s, c)] = seqp.tile([dw, TPC], f16, tag=f"seq{s}{c}",
                                           name=f"{pfx}sq_{s}{c}")
                    seq_eng[(s, c)].dma_start(sq[(s, c)][:],
                                              sqt[s][d0:d0 + dw, :])
            w_t = {}
            for c, (d0, dw) in enumerate(DCH):
                w_t[c] = cst.tile([dw, DH], f16, tag=f"w{c}", name=f"{pfx}w_t{c}")
                nc.scalar.dma_start(w_t[c][:], w_d[d0:d0 + dw, :])
            bias_t = cst.tile([DH, 1], f32, tag="bias", name=pfx + "bias_t")
            nc.scalar.dma_start(bias_t[:], bias_d[:])
            ident_t = cst.tile([DH, DH], f32, tag="ident", name=pfx + "ident_t")
            nc.scalar.dma_start(ident_t[:], ident_d[:])
            mbias_t = cst.tile([4 * RV, PAIRS * RL], f32, tag="mbias",
                               name=pfx + "mbias_t")
            nc.scalar.dma_start(mbias_t[:], mbias_d[:])

            taT, acc, mean, aoutT = {}, {}, {}, {}
            for s in "ab":
                taT[s] = bigp.tile([DH, TPC], f16, tag=f"taT{s}",
                                   name=f"{pfx}taT_{s}")
                acc[s] = cst.tile([DH, BPC], f32, tag=f"acc{s}", name=f"{pfx}acc_{s}")
                mean[s] = cst.tile([DH, BPC], f16, tag=f"mean{s}",
                                   name=f"{pfx}mean_{s}")
                aoutT[s] = cst.tile([DH, RPC], f16, tag=f"aoutT{s}",
                                    name=f"{pfx}aoutT_{s}")
            w2dall = cst.tile([4 * RV, PAIRS * RL], f32, tag="w2dall",
                              name=pfx + "w2dall")

            other = {"a": "b", "b": "a"}

            def emit_fc(smp):
                if stage < 1:
                    return
                t0 = smp * TPS
                pfc = {}
                for s in ("b", "a"):
                    pfc[s] = ps.tile([DH, TPS], f32, tag="fc", bufs=2,
                                     name=f"{pfx}pfc_{s}{smp}")
                # c-outer: 3 weight loads per sample instead of 18
                for c in range(3):
                    for s in ("b", "a"):
                        for n0, nw in NCH:
                            nc.tensor.matmul(
                                pfc[s][:, n0:n0 + nw],
                                w_t[c][:],
                                sq[(s, c)][:, t0 + n0:t0 + n0 + nw],
                                start=(c == 0), stop=(c == 2))
                for s in ("b", "a"):
                    nc.scalar.activation(
                        taT[s][:, t0:t0 + TPS], pfc[s][:], AF.Relu,
                        bias=bias_t[:], accum_out=acc[s][:, smp:smp + 1])
                    nc.scalar.mul(mean[s][:, smp:smp + 1],
                                  acc[s][:, smp:smp + 1], 1.0 / TPS)

            def emit_tail(q):
                """Scores + softmax + weighted sum for sample pair q
                (samples 2q, 2q+1).  Row/col order everywhere: (side i,
                pair-member k, review r) = i*20 + k*10 + r."""
                if stage < 2:
                    return
                # scores: per-review matvecs, taT block stationary, mean
                # moving; psT[:, col] = 128 token scores of one review.
                psT = ps.tile([DH, 4 * RV], f32, tag="sc", bufs=1,
                              name=f"{pfx}psT_{q}")
                for i, s in enumerate(("a", "b")):
                    for k in range(2):
                        smp = 2 * q + k
                        for r in range(RV):
                            col = i * 2 * RV + k * RV + r
                            blk = smp * TPS + r * RL
                            nc.tensor.matmul(
                                psT[:, col:col + 1],
                                taT[s][:, blk:blk + RL],
                                mean[other[s]][:, smp:smp + 1])
                scsT = smp_pool.tile([DH, 4 * RV], f32, tag="scsT", bufs=2,
                                     name=f"{pfx}scsT_{q}")
                nc.scalar.copy(scsT[:], psT[:])
                pscs = ps.tile([4 * RV, RL], f32, tag="tp", bufs=1,
                               name=f"{pfx}pscs_{q}")
                nc.tensor.matmul(pscs[:], scsT[:], ident_t[:],
                                 is_transpose=True)
                scs = smp_pool.tile([4 * RV, RL], f32, tag="scs", bufs=2,
                                    name=f"{pfx}scs_{q}")
                nc.scalar.copy(scs[:], pscs[:])

                # masked softmax, all 2 sides x 2 samples x 10 reviews at once
                lgs = smp_pool.tile([4 * RV, RL], f32, tag="lgs", bufs=2,
                                    name=f"{pfx}lgs_{q}")
                nc.vector.tensor_tensor(
                    out=lgs[:], in0=scs[:],
                    in1=mbias_t[:, q * RL:(q + 1) * RL],
                    op=mybir.AluOpType.add)
                negmax = smp_pool.tile([4 * RV, 1], f32, tag="negmax", bufs=2,
                                       name=f"{pfx}negmax_{q}")
                nc.vector.reduce_max(out=negmax[:], in_=lgs[:],
                                     axis=AX.X, negate=True)
                e2d = smp_pool.tile([4 * RV, RL], f32, tag="e2d", bufs=2,
                                    name=f"{pfx}e2d_{q}")
                ssum = smp_pool.tile([4 * RV, 1], f32, tag="ssum", bufs=2,
                                     name=f"{pfx}ssum_{q}")
                nc.scalar.activation(e2d[:], lgs[:], AF.Exp, bias=negmax[:],
                                     accum_out=ssum[:])
                rec = smp_pool.tile([4 * RV, 1], f32, tag="rec", bufs=2,
                                    name=f"{pfx}rec_{q}")
                nc.vector.reciprocal(rec[:], ssum[:])
                nc.vector.tensor_scalar_mul(
                    w2dall[:, q * RL:(q + 1) * RL], e2d[:], rec[:])
                w16 = smp_pool.tile([4 * RV, RL], f16, tag="w16", bufs=2,
                                    name=f"{pfx}w16_{q}")
                nc.vector.tensor_scalar_mul(w16[:], e2d[:], rec[:])

                # weighted sums: one broadcast + in-place multiply +
                # segmented reduce per side, covering both pair samples.
                for i, s in enumerate(("a", "b") if stage >= 3 else ()):
                    wflat = smp_pool.tile([1, TPP], f16, tag="wflat", bufs=4,
                                          name=f"{pfx}wflat_{s}{q}")
                    nc.sync.dma_start(
                        wflat[:], w16[i * 2 * RV:(i + 1) * 2 * RV, :])
                    wbc = smp_pool.tile([DH, TPP], f16, tag="wbc", bufs=3,
                                        name=f"{pfx}wbc_{s}{q}")
                    nc.gpsimd.partition_broadcast(wbc[:], wflat[:])
                    nc.vector.tensor_tensor(
                        out=wbc[:], in0=taT[s][:, q * TPP:(q + 1) * TPP],
                        in1=wbc[:], op=mybir.AluOpType.mult)
                    with nc.allow_low_precision(
                            reason="fp16 segmented sum of 128 bounded "
                            "terms; validated ~4e-3 vs fp32 reference"):
                        nc.vector.reduce_sum(
                            out=aoutT[s][:, q * 2 * RV:(q + 1) * 2 * RV],
                            in_=wbc[:].rearrange("p (g l) -> p g l", g=2 * RV),
                            axis=AX.X)

            # FC pipeline with the pair-tail trailing one sample behind
            emit_fc(0)
            emit_fc(1)
            emit_fc(2)
            emit_tail(0)
            emit_fc(3)
            emit_tail(1)

            # ---- per-side epilogue: weights out, transpose, vectors out
            for si, s in enumerate(("a", "b") if stage >= 2 else ()):
                nc.sync.dma_start(
                    out_w[s].rearrange("(q p) l -> p (q l)", q=PAIRS),
                    w2dall[si * 2 * RV:(si + 1) * 2 * RV, :])
                aoutT32 = smp_pool.tile([DH, RPC], f32, tag="aoutT32",
                                        name=f"{pfx}aoutT32_{s}")
                nc.vector.tensor_copy(aoutT32[:], aoutT[s][:])
                ptp = ps.tile([RPC, DH], f32, tag="tp", bufs=1,
                              name=f"{pfx}ptp_{s}")
                nc.tensor.matmul(ptp[:], aoutT32[:], ident_t[:],
                                 is_transpose=True)
                aout = smp_pool.tile([RPC, DH], f32, tag="aout",
                                     name=f"{pfx}aout_{s}")
                nc.vector.tensor_copy(aout[:], ptp[:])
                nc.sync.dma_start(out_v[s][:], aout[:])

    nc.compile()
    return nc


def build_in_maps(seq_a, seq_b, mask_a, mask_b, W, b):
    seq_a = np.asarray(seq_a, dtype=np.float32)
    seq_b = np.asarray(seq_b, dtype=np.float32)
    mask_a = np.asarray(mask_a, dtype=np.int32)
    mask_b = np.asarray(mask_b, dtype=np.int32)
    W = np.asarray(W, dtype=np.float32)
    b = np.asarray(b, dtype=np.float32)

    ident_np = np.eye(DH, dtype=np.float32)
    bias_np = np.ascontiguousarray(b.reshape(DH, 1))
    w_np = np.ascontiguousarray(W.astype(np.float16))

    in_maps = []
    for core in range(NCORES):
        b0 = core * BPC
        sl = {}
        for name, seq in (("a", seq_a), ("b", seq_b)):
            chunk = seq[b0:b0 + BPC].reshape(TPC, DIN)
            sl[f"sqt_{name}"] = np.ascontiguousarray(chunk.T.astype(np.float16))
        # additive mask bias, rows (side, pair-member, review), cols (pair, l)
        mb = np.empty((4 * RV, PAIRS * RL), dtype=np.float32)
        for i, mask in enumerate((mask_a, mask_b)):
            m = mask[b0:b0 + BPC].reshape(PAIRS, 2, RV, RL)  # q, k, r, l
            for q in range(PAIRS):
                for k in range(2):
                    mb[i * 2 * RV + k * RV:i * 2 * RV + (k + 1) * RV,
                       q * RL:(q + 1) * RL] = np.where(m[q, k] > 0, 0.0, NEG_INF)
        sl["mbias"] = mb
        sl["w"] = w_np
        sl["bias"] = bias_np
        sl["ident"] = ident_np
        in_maps.append(sl)
    return in_maps


def kernel(seq_a, seq_b, mask_a, mask_b, W, b):
    if "nc" not in _CACHE:
        _CACHE["nc"] = _build()
    nc = _CACHE["nc"]
    in_maps = build_in_maps(seq_a, seq_b, mask_a, mask_b, W, b)

    from concourse.bass_utils import run_bass_kernel_spmd
    res = run_bass_kernel_spmd(nc, in_maps, core_ids=list(range(NCORES)))
    _CACHE["last_res"] = res

    a_out = np.concatenate([r["out_a"] for r in res.results], axis=0)
    b_out = np.concatenate([r["out_b"] for r in res.results], axis=0)
    atob_w = np.concatenate([r["outw_a"] for r in res.results], axis=0)
    btoa_w = np.concatenate([r["outw_b"] for r in res.results], axis=0)
    return (a_out, b_out, atob_w, btoa_w)


# revision 17
# speedup vs baseline: 1.1382x; 1.1382x over previous
"""CoAttention kernel for Trainium2 (8 NeuronCores, data-parallel over batch).

Math (per sample): ta = relu(seq_a @ W + b), tb likewise.  The reference
mean-pools the [N, rv_len, M] affinity before softmax, and mean-pooling
commutes with the dot product:

    atob_scores[n, l] = mean_m( ta[n,l,:] . tb_all_tokens[m,:] )
                      = ta[n,l,:] . mean_m( tb_all_tokens[m,:] )

so each side only needs a dot with the *other side's per-sample mean
feature vector* — the 52M-element affinity tensor is never materialized.

Memory regime: HBM-bound on the seq reads, so the host pre-converts seq
(and W) to fp16, halving DMA traffic; end-to-end error stays ~4e-3 vs
the fp32 reference.  Seq streams in as 6 full-span DMAs per iteration
(one per side x contraction-chunk) split across the two HWDGE queues
(SP + Act); gpsimd's software DGE issues nothing.

The FC runs as fp16 PE matmuls with W stationary; taT [hdim, tokens]
stays resident in SBUF as fp16 (enables the DVE 2x perf mode on the
weighted-sum multiply).  Scores are per-review matvecs with the taT
block stationary and the other side's mean as a 1-col moving operand,
landing transposed in a [128, 40] PSUM tile -> free-size-40 act-engine
evacuation + one PE transpose restores the softmax layout.

The tail (scores/softmax/weighted-sum) processes SAMPLE PAIRS: all
softmax ops run on [40,128] tiles, masking is a single tensor_tensor
add of a host-prebuilt additive bias (0 / -1e9), and the weighted sum
does one [128,2560] broadcast+in-place-multiply+segmented-reduce per
side per pair — halving tail instruction count vs per-sample tiles.
"""
import sys

sys.path.insert(0, "/opt/trn_rl_repo")

import numpy as np

import concourse.bacc as bacc
import concourse.tile as tile
from concourse import mybir

# Problem shape (hardcoded per contest contract)
BZ, RV, RL, DIN, DH = 32, 10, 128, 300, 128
NCORES = 8
BPC = BZ // NCORES            # samples per core: 4
PAIRS = BPC // 2              # tail processes samples in pairs: 2
TPC = BPC * RV * RL           # tokens per core per side: 5120
TPS = RV * RL                 # tokens per sample: 1280
TPP = 2 * TPS                 # tokens per pair: 2560
RPC = BPC * RV                # reviews per core: 40
NEG_INF = -1e9

f32 = mybir.dt.float32
f16 = mybir.dt.float16
i32 = mybir.dt.int32
AF = mybir.ActivationFunctionType
AX = mybir.AxisListType

# d-chunks of the contraction dim (K <= 128)
DCH = [(0, 128), (128, 128), (256, 44)]
# free-dim chunks of one sample's tokens (N <= 512 per PSUM bank)
NCH = [(0, 512), (512, 512), (1024, 256)]

_CACHE = {}


def _build(iters=1, serial=False, loop_n=0, stage=3, dma_split=0):
    nc = bacc.Bacc("TRN2", target_bir_lowering=False, debug=False)

    sqt = {s: nc.dram_tensor(f"sqt_{s}", [DIN, TPC], f16, kind="ExternalInput")
           for s in "ab"}
    mbias_d = nc.dram_tensor("mbias", [4 * RV, PAIRS * RL], f32,
                             kind="ExternalInput")
    w_d = nc.dram_tensor("w", [DIN, DH], f16, kind="ExternalInput")
    bias_d = nc.dram_tensor("bias", [DH, 1], f32, kind="ExternalInput")
    ident_d = nc.dram_tensor("ident", [DH, DH], f32, kind="ExternalInput")

    out_v = {s: nc.dram_tensor(f"out_{s}", [RPC, DH], f32, kind="ExternalOutput")
             for s in "ab"}
    out_w = {s: nc.dram_tensor(f"outw_{s}", [RPC, RL], f32, kind="ExternalOutput")
             for s in "ab"}

    import contextlib
    outer_tc = tile.TileContext(nc) if not serial else None
    with (outer_tc if outer_tc is not None else contextlib.nullcontext()):
      for it_ in range(iters):
        pfx = f"i{it_}_" if iters > 1 else ""
        with (
            tile.TileContext(nc) if serial else contextlib.nullcontext()
        ) as maybe_tc:
          tc = maybe_tc if serial else outer_tc
          with (
            tc.For_i(0, loop_n, 1) if loop_n else contextlib.nullcontext()
          ):
           with (
            tc.tile_pool(name=pfx + "cst", bufs=1) as cst,
            tc.tile_pool(name=pfx + "seq", bufs=12) as seqp,
            tc.tile_pool(name=pfx + "big", bufs=1) as bigp,
            tc.tile_pool(name=pfx + "sm", bufs=2) as smp_pool,
            tc.tile_pool(name=pfx + "ps", bufs=2, space="PSUM") as ps,
        ):
            w_t = {}
            for c, (d0, dw) in enumerate(DCH):
                w_t[c] = cst.tile([dw, DH], f16, tag=f"w{c}", name=f"{pfx}w_t{c}")
                nc.gpsimd.dma_start(w_t[c][:], w_d[d0:d0 + dw, :])
            bias_t = cst.tile([DH, 1], f32, tag="bias", name=pfx + "bias_t")
            nc.gpsimd.dma_start(bias_t[:], bias_d[:])
            # late-needed constants go on gpsimd so SP can stream seq tiles
            ident_t = cst.tile([DH, DH], f32, tag="ident", name=pfx + "ident_t")
            nc.gpsimd.dma_start(ident_t[:], ident_d[:])
            mbias_t = cst.tile([4 * RV, PAIRS * RL], f32, tag="mbias",
                               name=pfx + "mbias_t")
            nc.gpsimd.dma_start(mbias_t[:], mbias_d[:])

            taT, acc, mean, aoutT = {}, {}, {}, {}
            for s in "ab":
                taT[s] = bigp.tile([DH, TPC], f16, tag=f"taT{s}",
                                   name=f"{pfx}taT_{s}")
                acc[s] = cst.tile([DH, BPC], f32, tag=f"acc{s}", name=f"{pfx}acc_{s}")
                mean[s] = cst.tile([DH, BPC], f16, tag=f"mean{s}",
                                   name=f"{pfx}mean_{s}")
                aoutT[s] = cst.tile([DH, RPC], f16, tag=f"aoutT{s}",
                                    name=f"{pfx}aoutT_{s}")
            w2dall = cst.tile([4 * RV, PAIRS * RL], f32, tag="w2dall",
                              name=pfx + "w2dall")

            other = {"a": "b", "b": "a"}

            sq = {}

            def emit_fc(smp):
                t0 = smp * TPS
                dma_eng = ([nc.sync, nc.sync, nc.gpsimd] if dma_split == 0
                           else [nc.sync, nc.scalar, nc.gpsimd])
                for s in ("b", "a"):
                    for c, (d0, dw) in enumerate(DCH):
                        sq[(s, c, smp)] = seqp.tile(
                            [dw, TPS], f16, tag="seq",
                            name=f"{pfx}sq_{s}{smp}{c}")
                        dma_eng[c].dma_start(sq[(s, c, smp)][:],
                                             sqt[s][d0:d0 + dw, t0:t0 + TPS])
                if stage < 1:
                    return
                pfc = {}
                for s in ("b", "a"):
                    pfc[s] = ps.tile([DH, TPS], f32, tag="fc", bufs=2,
                                     name=f"{pfx}pfc_{s}{smp}")
                # c-outer: 3 weight loads per sample instead of 18
                for c in range(3):
                    for s in ("b", "a"):
                        for n0, nw in NCH:
                            nc.tensor.matmul(
                                pfc[s][:, n0:n0 + nw],
                                w_t[c][:],
                                sq[(s, c, smp)][:, n0:n0 + nw],
                                start=(c == 0), stop=(c == 2))
                for s in ("b", "a"):
                    nc.scalar.activation(
                        taT[s][:, t0:t0 + TPS], pfc[s][:], AF.Relu,
                        bias=bias_t[:], accum_out=acc[s][:, smp:smp + 1])
                    nc.scalar.mul(mean[s][:, smp:smp + 1],
                                  acc[s][:, smp:smp + 1], 1.0 / TPS)

            def emit_tail(q):
                """Scores + softmax + weighted sum for sample pair q
                (samples 2q, 2q+1).  Row/col order everywhere: (side i,
                pair-member k, review r) = i*20 + k*10 + r."""
                if stage < 2:
                    return
                # scores: per-review matvecs, taT block stationary, mean
                # moving; psT[:, col] = 128 token scores of one review.
                psT = ps.tile([DH, 4 * RV], f32, tag="sc", bufs=1,
                              name=f"{pfx}psT_{q}")
                for i, s in enumerate(("a", "b")):
                    for k in range(2):
                        smp = 2 * q + k
                        for r in range(RV):
                            col = i * 2 * RV + k * RV + r
                            blk = smp * TPS + r * RL
                            nc.tensor.matmul(
                                psT[:, col:col + 1],
                                taT[s][:, blk:blk + RL],
                                mean[other[s]][:, smp:smp + 1])
                scsT = smp_pool.tile([DH, 4 * RV], f32, tag="scsT", bufs=2,
                                     name=f"{pfx}scsT_{q}")
                nc.scalar.copy(scsT[:], psT[:])
                pscs = ps.tile([4 * RV, RL], f32, tag="tp", bufs=1,
                               name=f"{pfx}pscs_{q}")
                nc.tensor.matmul(pscs[:], scsT[:], ident_t[:],
                                 is_transpose=True)
                scs = smp_pool.tile([4 * RV, RL], f32, tag="scs", bufs=2,
                                    name=f"{pfx}scs_{q}")
                nc.scalar.copy(scs[:], pscs[:])

                # masked softmax, all 2 sides x 2 samples x 10 reviews at once
                lgs = smp_pool.tile([4 * RV, RL], f32, tag="lgs", bufs=2,
                                    name=f"{pfx}lgs_{q}")
                nc.vector.tensor_tensor(
                    out=lgs[:], in0=scs[:],
                    in1=mbias_t[:, q * RL:(q + 1) * RL],
                    op=mybir.AluOpType.add)
                negmax = smp_pool.tile([4 * RV, 1], f32, tag="negmax", bufs=2,
                                       name=f"{pfx}negmax_{q}")
                nc.vector.reduce_max(out=negmax[:], in_=lgs[:],
                                     axis=AX.X, negate=True)
                e2d = smp_pool.tile([4 * RV, RL], f32, tag="e2d", bufs=2,
                                    name=f"{pfx}e2d_{q}")
                ssum = smp_pool.tile([4 * RV, 1], f32, tag="ssum", bufs=2,
                                     name=f"{pfx}ssum_{q}")
                nc.scalar.activation(e2d[:], lgs[:], AF.Exp, bias=negmax[:],
                                     accum_out=ssum[:])
                rec = smp_pool.tile([4 * RV, 1], f32, tag="rec", bufs=2,
                                    name=f"{pfx}rec_{q}")
                nc.vector.reciprocal(rec[:], ssum[:])
                nc.vector.tensor_scalar_mul(
                    w2dall[:, q * RL:(q + 1) * RL], e2d[:], rec[:])
                w16 = smp_pool.tile([4 * RV, RL], f16, tag="w16", bufs=2,
                                    name=f"{pfx}w16_{q}")
                nc.vector.tensor_scalar_mul(w16[:], e2d[:], rec[:])

                # weighted sums: one broadcast + in-place multiply +
                # segmented reduce per side, covering both pair samples.
                for i, s in enumerate(("a", "b") if stage >= 3 else ()):
                    wflat = smp_pool.tile([1, TPP], f16, tag="wflat", bufs=4,
                                          name=f"{pfx}wflat_{s}{q}")
                    nc.sync.dma_start(
                        wflat[:], w16[i * 2 * RV:(i + 1) * 2 * RV, :])
                    wbc = smp_pool.tile([DH, TPP], f16, tag="wbc", bufs=3,
                                        name=f"{pfx}wbc_{s}{q}")
                    nc.gpsimd.partition_broadcast(wbc[:], wflat[:])
                    nc.vector.tensor_tensor(
                        out=wbc[:], in0=taT[s][:, q * TPP:(q + 1) * TPP],
                        in1=wbc[:], op=mybir.AluOpType.mult)
                    with nc.allow_low_precision(
                            reason="fp16 segmented sum of 128 bounded "
                            "terms; validated ~4e-3 vs fp32 reference"):
                        nc.vector.reduce_sum(
                            out=aoutT[s][:, q * 2 * RV:(q + 1) * 2 * RV],
                            in_=wbc[:].rearrange("p (g l) -> p g l", g=2 * RV),
                            axis=AX.X)

            # FC pipeline with the pair-tail trailing one sample behind
            emit_fc(0)
            emit_fc(1)
            emit_fc(2)
            emit_tail(0)
            emit_fc(3)
            emit_tail(1)

            # ---- per-side epilogue: weights out, transpose, vectors out
            for si, s in enumerate(("a", "b") if stage >= 2 else ()):
                nc.sync.dma_start(
                    out_w[s].rearrange("(q p) l -> p q l", q=PAIRS),
                    w2dall[si * 2 * RV:(si + 1) * 2 * RV, :]
                    .rearrange("p (q l) -> p q l", q=PAIRS))
                aoutT32 = smp_pool.tile([DH, RPC], f32, tag="aoutT32",
                                        name=f"{pfx}aoutT32_{s}")
                nc.vector.tensor_copy(aoutT32[:], aoutT[s][:])
                ptp = ps.tile([RPC, DH], f32, tag="tp", bufs=1,
                              name=f"{pfx}ptp_{s}")
                nc.tensor.matmul(ptp[:], aoutT32[:], ident_t[:],
                                 is_transpose=True)
                aout = smp_pool.tile([RPC, DH], f32, tag="aout",
                                     name=f"{pfx}aout_{s}")
                nc.vector.tensor_copy(aout[:], ptp[:])
                nc.sync.dma_start(out_v[s][:], aout[:])

    nc.compile()
    return nc


def build_in_maps(seq_a, seq_b, mask_a, mask_b, W, b):
    seq_a = np.asarray(seq_a, dtype=np.float32)
    seq_b = np.asarray(seq_b, dtype=np.float32)
    mask_a = np.asarray(mask_a, dtype=np.int32)
    mask_b = np.asarray(mask_b, dtype=np.int32)
    W = np.asarray(W, dtype=np.float32)
    b = np.asarray(b, dtype=np.float32)

    ident_np = np.eye(DH, dtype=np.float32)
    bias_np = np.ascontiguousarray(b.reshape(DH, 1))
    w_np = np.ascontiguousarray(W.astype(np.float16))

    in_maps = []
    for core in range(NCORES):
        b0 = core * BPC
        sl = {}
        for name, seq in (("a", seq_a), ("b", seq_b)):
            chunk = seq[b0:b0 + BPC].reshape(TPC, DIN)
            sl[f"sqt_{name}"] = np.ascontiguousarray(chunk.T.astype(np.float16))
        # additive mask bias, rows (side, pair-member, review), cols (pair, l)
        mb = np.empty((4 * RV, PAIRS * RL), dtype=np.float32)
        for i, mask in enumerate((mask_a, mask_b)):
            m = mask[b0:b0 + BPC].reshape(PAIRS, 2, RV, RL)  # q, k, r, l
            for q in range(PAIRS):
                for k in range(2):
                    mb[i * 2 * RV + k * RV:i * 2 * RV + (k + 1) * RV,
                       q * RL:(q + 1) * RL] = np.where(m[q, k] > 0, 0.0, NEG_INF)
        sl["mbias"] = mb
        sl["w"] = w_np
        sl["bias"] = bias_np
        sl["ident"] = ident_np
        in_maps.append(sl)
    return in_maps


def kernel(seq_a, seq_b, mask_a, mask_b, W, b):
    if "nc" not in _CACHE:
        _CACHE["nc"] = _build()
    nc = _CACHE["nc"]
    in_maps = build_in_maps(seq_a, seq_b, mask_a, mask_b, W, b)

    from concourse.bass_utils import run_bass_kernel_spmd
    res = run_bass_kernel_spmd(nc, in_maps, core_ids=list(range(NCORES)))
    _CACHE["last_res"] = res

    a_out = np.concatenate([r["out_a"] for r in res.results], axis=0)
    b_out = np.concatenate([r["out_b"] for r in res.results], axis=0)
    atob_w = np.concatenate([r["outw_a"] for r in res.results], axis=0)
    btoa_w = np.concatenate([r["outw_b"] for r in res.results], axis=0)
    return (a_out, b_out, atob_w, btoa_w)
